# revision 12
# baseline (speedup 1.0000x reference)
"""BarPooling kernel for 8 Trainium2 NeuronCores.

Computes, for beat_emb [B=8, M=8192, D=1024], W [1024, 1056], b [1024]:
    mean = segment_mean(beat_emb, K=4)            # [B, 2048, 1024]
    h    = concat([mean, fourier(pos)], -1)       # [B, 2048, 1056]
    out  = h @ W.T + b                            # [B, 2048, 1024]

Sharding: data-parallel over B (core i handles batch i); W replicated.

Per-core device pipeline (v2 — PE-pooling, bf16 GEMM):
  1. DMA x in [128 beats, 4*1024] tiles (4 beat-subtiles of 128).
  2. PE pooling matmul per (subtile, d-chunk): lhsT = x subtile chunk
     [128 beats, 128 d] (stationary), rhs = P [128 beats, 32 bars]
     (0/1 pooling matrix) -> PSUM [128 d, 32 bars].  This produces the
     segment sums ALREADY TRANSPOSED for the main GEMM - no DVE adds,
     no PE transposes, no identity matrix.
  3. Act-engine copies PSUM -> SBUF mts slabs, casting fp32 -> bf16.
  4. PE main matmul (bf16): out[bars, o] = sum_ic mts[ic].T @ WT[ic] with
     the fourier+bias tail folded in as an extra 33-deep accumulation.
  5. DVE copies PSUM -> SBUF fp32, out DMA on the Act HWDGE queue.

All constants are bf16 (W pre-scaled by 0.25 = the /4 segment mean; the
pooling matrix P is fp32 bit-packed into bf16 pairs).  Two const DMAs:
small (P+fourier+tail) first so the PE warmup fires early, big WT second
overlapping the first x tiles.
"""

import math
from contextlib import ExitStack

import numpy as np
import ml_dtypes

import concourse.bass as bass
import concourse.bacc as bacc
import concourse.mybir as mybir
import concourse.tile as tile
from concourse.bass_utils import run_bass_kernel_spmd

B, M, D = 8, 8192, 1024
KBEATS = 4
POS = 32
MB = M // KBEATS          # 2048 bars
NCORES = 8
ICH = D // 128            # 8 contraction chunks of 128
NGRP = 16                 # groups of 512 beats -> 128 bars
JSUB = 4                  # beat-subtiles of 128 per group

# packed constant tensor column layout (one [128, CST_F] bf16 tensor)
COL_WT = 0                    # 8 chunks of [128, 1024]: 0.25*W1T rows ic*128..+128
COL_WTAIL = 8 * D             # [33, 1024]: [W2T; b]
COL_FFT = COL_WTAIL + D       # [33, 2048]: [fourierT; ones]
COL_P = COL_FFT + MB          # [128, 256]: fp32 P [128,128] bit-packed
CST_F = COL_P + 256


def _fourier_T() -> np.ndarray:
    """[33, 2048]: rows 0..31 = fourier features (transposed), row 32 = ones."""
    half = POS // 2
    freqs = np.exp(np.linspace(0.0, math.log(1000.0), half)).astype(np.float32)
    idx = np.arange(MB, dtype=np.float32)
    pos = np.clip(idx / np.float32(MB - 1), 0.0, 1.0).astype(np.float32)
    ang = pos[:, None] * freqs[None, :]
    ff = np.concatenate([np.sin(ang), np.cos(ang)], axis=1).astype(np.float32)
    return np.concatenate([ff.T, np.ones((1, MB), np.float32)], axis=0)


def _emit(nc: bass.Bass, reps: int = 1) -> None:
    f32 = mybir.dt.float32
    bf16 = mybir.dt.bfloat16
    x = nc.declare_dram_parameter("x", [M, D], f32, isOutput=False)
    cst = nc.declare_dram_parameter("cst", [128, CST_F], bf16, isOutput=False)
    # tok/otok: tiny passthrough used by the benchmark harness; ~zero cost.
    tok = nc.declare_dram_parameter("tok", [128, 128], f32, isOutput=False)
    out = nc.declare_dram_parameter("out", [MB, D], f32, isOutput=True)
    otok = nc.declare_dram_parameter("otok", [128, 128], f32, isOutput=True)

    with tile.TileContext(nc) as tc, ExitStack() as ctx:
        const = ctx.enter_context(tc.tile_pool(name="const", bufs=1))
        xpool = ctx.enter_context(tc.tile_pool(name="xp", bufs=3))
        mtpool = ctx.enter_context(tc.tile_pool(name="mtp", bufs=2))
        opool = ctx.enter_context(tc.tile_pool(name="ob", bufs=3))
        ppool = ctx.enter_context(tc.tile_pool(name="pp", bufs=2, space="PSUM"))
        pmm = ctx.enter_context(tc.tile_pool(name="pmm", bufs=2, space="PSUM"))

        cst_sb = const.tile([128, CST_F], bf16, tag="cst")
        # consts on the Act HWDGE queue so x tile 0 starts on SP at t=0;
        # small consts first (P + fourier + tail rows 0..33), big WT second
        nc.scalar.dma_start(out=cst_sb[:, COL_P:], in_=cst[:, COL_P:])
        nc.scalar.dma_start(
            out=cst_sb[0:POS + 1, COL_WTAIL:COL_P],
            in_=cst[0:POS + 1, COL_WTAIL:COL_P],
        )
        nc.scalar.dma_start(
            out=cst_sb[:, COL_WT:COL_WTAIL], in_=cst[:, COL_WT:COL_WTAIL]
        )
        nc.scalar.dma_start(out=otok[:, :], in_=tok[:, :])
        pmat = cst_sb[:, COL_P:COL_P + 256].bitcast(f32)  # [128, 128] fp32

        def wt_slice(ic, oc):
            return cst_sb[:, COL_WT + ic * D + oc * 512:COL_WT + ic * D + (oc + 1) * 512]

        def wtail_slice(oc):
            return cst_sb[0:POS + 1, COL_WTAIL + oc * 512:COL_WTAIL + (oc + 1) * 512]

        def fft_slice(gm):
            return cst_sb[0:POS + 1, COL_FFT + gm * 128:COL_FFT + (gm + 1) * 128]

        # PE warmup: consumes the small-const DMA sem so later pooling
        # matmuls wait only on their x-tile DMA (walrus: one sem-wait max
        # on LDWEIGHTS).  Output is scratch.
        ps_warm = pmm.tile([128, 32], f32, tag="pm0")
        nc.tensor.matmul(
            ps_warm[:], lhsT=pmat[:, 0:128], rhs=pmat[:, 0:32],
            start=True, stop=True,
        )

        # [16 groups, 4 subtiles, 128 beats, 1024] view: partition = beat
        # within subtile, free dims (subtile, d)
        xv = x.rearrange("(g j p) d -> g p j d", j=JSUB, p=128)

        def body():
            for g in range(NGRP):
                xt = xpool.tile([128, JSUB * D], f32, tag="xt")
                # alternate x tiles across both HWDGE queues so the load
                # chain isn't serialized behind a single queue
                xeng = nc.sync if g % 2 == 0 else nc.scalar
                xeng.dma_start(
                    out=xt.rearrange("p (j d) -> p j d", d=D), in_=xv[g]
                )

                # pooling matmuls: [128 d, 32 bars] strips into 2 PSUM banks
                pps = [
                    ppool.tile([128, 512], f32, tag=f"pp{h}", name=f"pp{h}")
                    for h in range(2)
                ]
                for ic in range(ICH):
                    for j in range(JSUB):
                        nc.tensor.matmul(
                            pps[ic // 4][:, (ic % 4) * 128 + j * 32:
                                         (ic % 4) * 128 + (j + 1) * 32],
                            lhsT=xt[:, j * D + ic * 128:j * D + (ic + 1) * 128],
                            rhs=pmat[:, 0:32],
                            start=True,
                            stop=True,
                        )

                # PSUM -> SBUF bf16 slabs (Act engine; casts fp32->bf16)
                mts = []
                for ic in range(ICH):
                    mt = mtpool.tile([128, 128], bf16, tag=f"mt{ic}")
                    nc.scalar.copy(
                        out=mt[:],
                        in_=pps[ic // 4][:, (ic % 4) * 128:(ic % 4 + 1) * 128],
                    )
                    mts.append(mt)

                # main GEMM: bars g*128..+128, all 1024 outputs in 2 halves
                osb = opool.tile([128, D], f32, tag="osb")
                for oc in range(2):
                    pm = pmm.tile([128, 512], f32, tag=f"pm{oc}")
                    for ic in range(ICH):
                        nc.tensor.matmul(
                            pm[:],
                            lhsT=mts[ic][:],
                            rhs=wt_slice(ic, oc),
                            start=(ic == 0),
                            stop=False,
                        )
                    nc.tensor.matmul(
                        pm[:],
                        lhsT=fft_slice(g),
                        rhs=wtail_slice(oc),
                        start=False,
                        stop=True,
                    )
                    nc.vector.tensor_copy(osb[:, oc * 512:(oc + 1) * 512], pm[:])
                # out writes on the gpsimd SWDGE queue (third DMA path)
                nc.gpsimd.dma_start(
                    out=out[g * 128:(g + 1) * 128, :], in_=osb[:]
                )

        if reps == 1:
            body()
        else:
            # hardware loop: reps iterations without reps x instructions
            # (benchmark-only path; kernel() always uses reps=1)
            with tc.For_i(0, reps):
                body()


_NC_CACHE: bass.Bass | None = None


def _get_nc() -> bass.Bass:
    global _NC_CACHE
    if _NC_CACHE is None:
        nc = bacc.Bacc(trn_type="TRN2")
        _emit(nc)
        nc.compile()
        _NC_CACHE = nc
    return _NC_CACHE


def _host_cst(W: np.ndarray, b: np.ndarray) -> np.ndarray:
    """Pack [0.25*W1T ; W2T ; b ; fourierT ; ones ; P] into one bf16 tensor."""
    wt_aug = np.concatenate(
        [
            0.25 * np.ascontiguousarray(W[:, :D].T),
            np.ascontiguousarray(W[:, D:].T),
            np.asarray(b, np.float32)[None, :],
        ],
        axis=0,
    ).astype(np.float32)  # [1057, 1024]
    fft = _fourier_T()

    cst = np.zeros((128, CST_F), dtype=ml_dtypes.bfloat16)
    for ic in range(ICH):
        cst[:, COL_WT + ic * D:COL_WT + (ic + 1) * D] = wt_aug[
            ic * 128:(ic + 1) * 128
        ].astype(ml_dtypes.bfloat16)
    cst[0:POS + 1, COL_WTAIL:COL_WTAIL + D] = wt_aug[D:].astype(ml_dtypes.bfloat16)
    cst[0:POS + 1, COL_FFT:COL_FFT + MB] = fft.astype(ml_dtypes.bfloat16)
    # P [128, 128] fp32, bit-packed into 256 bf16 columns
    pm = np.zeros((128, 128), np.float32)
    pm[np.arange(128), np.arange(128) // KBEATS] = 1.0
    cst[:, COL_P:COL_P + 256] = pm.view(ml_dtypes.bfloat16)
    return cst


def _host_inputs(beat_emb: np.ndarray, W: np.ndarray, b: np.ndarray):
    cst = _host_cst(np.asarray(W, np.float32), np.asarray(b, np.float32))
    tok = np.zeros((128, 128), np.float32)
    return [
        {
            "x": np.ascontiguousarray(beat_emb[i], dtype=np.float32),
            "cst": cst,
            "tok": tok,
        }
        for i in range(NCORES)
    ]


def _numpy_model(beat_emb: np.ndarray, W: np.ndarray, b: np.ndarray) -> np.ndarray:
    """Pure-numpy model of what the device computes (fast math check)."""
    cst = _host_cst(np.asarray(W, np.float32), np.asarray(b, np.float32))
    wt = np.concatenate(
        [
            np.concatenate(
                [
                    cst[:, COL_WT + ic * D:COL_WT + (ic + 1) * D]
                    for ic in range(ICH)
                ],
                axis=0,
            ),
            cst[0:POS + 1, COL_WTAIL:COL_WTAIL + D],
        ],
        axis=0,
    ).astype(np.float64)  # [1057, 1024]
    fft = cst[0:POS + 1, COL_FFT:COL_FFT + MB].astype(np.float64)
    outs = []
    for i in range(NCORES):
        x = np.asarray(beat_emb[i], np.float32).astype(np.float64)
        sums = x.reshape(MB, KBEATS, D).sum(axis=1)
        sums_b = sums.astype(np.float32).astype(ml_dtypes.bfloat16).astype(np.float64)
        h = np.concatenate([sums_b, fft.T], axis=1)  # [MB, 1057]
        outs.append((h @ wt).astype(np.float32))
    return np.stack(outs, 0)


def kernel(beat_emb: np.ndarray, W: np.ndarray, b: np.ndarray) -> np.ndarray:
    nc = _get_nc()
    in_maps = _host_inputs(np.asarray(beat_emb), np.asarray(W), np.asarray(b))
    res = run_bass_kernel_spmd(nc, in_maps, core_ids=list(range(NCORES)))
    return np.stack([np.asarray(res.results[i]["out"]) for i in range(NCORES)], axis=0)


# revision 13
# speedup vs baseline: 1.8913x; 1.8913x over previous
"""BarPooling kernel for 8 Trainium2 NeuronCores.

Computes, for beat_emb [B=8, M=8192, D=1024], W [1024, 1056], b [1024]:
    mean = segment_mean(beat_emb, K=4)            # [B, 2048, 1024]
    h    = concat([mean, fourier(pos)], -1)       # [B, 2048, 1056]
    out  = h @ W.T + b                            # [B, 2048, 1024]

Sharding: data-parallel over B (core i handles batch i); W replicated.

Per-core device pipeline (v3 — HW-trace-calibrated engine balance):
  1. x tiles [128 bars, 4*1024] fp32, 16 DMAs alternating the two HWDGE
     queues (SP/Act).  x load is HBM-bound (~98us) and is the wall to
     hide everything else under.
  2. DVE pairwise adds -> segment sums, cast to bf16 on the second add
     (~51us, hides under the x DMA).
  3. PE transposes sums chunks (bf16, 1 cycle/row) -> PSUM.
  4. Act-engine copies PSUM -> SBUF bf16 mts slabs.
  5. PE main GEMM (bf16): 18 matmuls per 128-bar group, fourier+bias
     tail folded in as an extra 33-deep accumulation.
  6. DVE copies PSUM -> SBUF fp32; out DMA on the gpsimd SWDGE queue.

Measured on HW (per 128-bar-group pipeline, microbench.py): x-load
98us, main GEMM+out 89us, transposes+copies 38us, DVE adds ~free under
DMA.  PE instruction count is the scarce resource on real HW (LDWEIGHTS
~237ns, small matmuls ~400ns regardless of size; PE throttles to ~50%
util under sustained load) — so pooling stays OFF the PE (v2's
PE-pooling with 512 tiny matmuls measured 273us for the compute chain).

All constants are bf16 (W pre-scaled by 0.25 = the /4 segment mean);
one [128, CST_F] bf16 tensor on the Act queue: small part (identity +
fourier + tail) first so the PE warmup fires early, big WT second
overlapping the first x tiles.
"""

import math
from contextlib import ExitStack

import numpy as np
import ml_dtypes

import concourse.bass as bass
import concourse.bacc as bacc
import concourse.mybir as mybir
import concourse.tile as tile
from concourse.bass_utils import run_bass_kernel_spmd

B, M, D = 8, 8192, 1024
KBEATS = 4
POS = 32
MB = M // KBEATS          # 2048 bars
NCORES = 8
ICH = D // 128            # 8 contraction chunks of 128
NGRP = 16                 # groups of 512 beats -> 128 bars

# packed constant tensor column layout (one [128, CST_F] bf16 tensor)
COL_WT = 0                    # 8 chunks of [128, 1024]: 0.25*W1T rows ic*128..+128
COL_WTAIL = 8 * D             # [33, 1024]: [W2T; b]
COL_FFT = COL_WTAIL + D       # [33, 2048]: [fourierT; ones]
COL_ID = COL_FFT + MB         # [128, 128] bf16 identity
CST_F = COL_ID + 128


def _fourier_T() -> np.ndarray:
    """[33, 2048]: rows 0..31 = fourier features (transposed), row 32 = ones."""
    half = POS // 2
    freqs = np.exp(np.linspace(0.0, math.log(1000.0), half)).astype(np.float32)
    idx = np.arange(MB, dtype=np.float32)
    pos = np.clip(idx / np.float32(MB - 1), 0.0, 1.0).astype(np.float32)
    ang = pos[:, None] * freqs[None, :]
    ff = np.concatenate([np.sin(ang), np.cos(ang)], axis=1).astype(np.float32)
    return np.concatenate([ff.T, np.ones((1, MB), np.float32)], axis=0)


def _emit(nc: bass.Bass, reps: int = 1) -> None:
    f32 = mybir.dt.float32
    bf16 = mybir.dt.bfloat16
    x = nc.declare_dram_parameter("x", [M, D], f32, isOutput=False)
    cst = nc.declare_dram_parameter("cst", [128, CST_F], bf16, isOutput=False)
    # tok/otok: tiny passthrough used by the benchmark harness; ~zero cost.
    tok = nc.declare_dram_parameter("tok", [128, 128], f32, isOutput=False)
    out = nc.declare_dram_parameter("out", [MB, D], f32, isOutput=True)
    otok = nc.declare_dram_parameter("otok", [128, 128], f32, isOutput=True)

    with tile.TileContext(nc) as tc, ExitStack() as ctx:
        const = ctx.enter_context(tc.tile_pool(name="const", bufs=1))
        xpool = ctx.enter_context(tc.tile_pool(name="xp", bufs=3))
        tpool = ctx.enter_context(tc.tile_pool(name="tp", bufs=2))
        spool = ctx.enter_context(tc.tile_pool(name="sp", bufs=3))
        mtpool = ctx.enter_context(tc.tile_pool(name="mtp", bufs=2))
        opool = ctx.enter_context(tc.tile_pool(name="ob", bufs=3))
        ptr = ctx.enter_context(tc.tile_pool(name="ptr", bufs=2, space="PSUM"))
        pmm = ctx.enter_context(tc.tile_pool(name="pmm", bufs=2, space="PSUM"))

        cst_sb = const.tile([128, CST_F], bf16, tag="cst")
        # consts on the Act HWDGE queue so x tile 0 starts on SP at t=0;
        # small consts (identity + fourier + tail) first, big WT second
        nc.scalar.dma_start(out=cst_sb[:, COL_ID:], in_=cst[:, COL_ID:])
        nc.scalar.dma_start(
            out=cst_sb[0:POS + 1, COL_WTAIL:COL_ID],
            in_=cst[0:POS + 1, COL_WTAIL:COL_ID],
        )
        nc.scalar.dma_start(
            out=cst_sb[:, COL_WT:COL_WTAIL], in_=cst[:, COL_WT:COL_WTAIL]
        )
        nc.scalar.dma_start(out=otok[:, :], in_=tok[:, :])
        ident = cst_sb[:, COL_ID:COL_ID + 128]

        def wt_slice(ic, oc):
            return cst_sb[:, COL_WT + ic * D + oc * 512:COL_WT + ic * D + (oc + 1) * 512]

        def wtail_slice(oc):
            return cst_sb[0:POS + 1, COL_WTAIL + oc * 512:COL_WTAIL + (oc + 1) * 512]

        def fft_slice(gm):
            return cst_sb[0:POS + 1, COL_FFT + gm * 128:COL_FFT + (gm + 1) * 128]

        # PE warmups: consume the two cst DMA sems so later LDWEIGHTS wait
        # only on their compute producers (walrus: one sem-wait max).
        ps_w1 = ptr.tile([128, 128], bf16, tag="ps0")
        nc.tensor.transpose(ps_w1[:], ident, ident)
        ps_w2 = pmm.tile([128, 32], f32, tag="pm0")
        nc.tensor.matmul(ps_w2[:], lhsT=ident, rhs=wt_slice(0, 0)[:, 0:32],
                         start=True, stop=True)

        # [16 tiles, 128 bars, 4*1024] view: 16KB contiguous per partition
        xv = x.rearrange("(t p k) d -> t p (k d)", p=128, k=KBEATS)

        def body():
            for g in range(NGRP):
                xt = xpool.tile([128, KBEATS * D], f32, tag="xt")
                # alternate x tiles across both HWDGE queues
                xeng = nc.sync if g % 2 == 0 else nc.scalar
                xeng.dma_start(out=xt, in_=xv[g])

                # DVE pairwise adds; second add casts to bf16
                xg = xt.rearrange("p (k2 j d) -> p k2 j d", j=2, d=D)
                tmp = tpool.tile([128, 2 * D], f32, tag="tmp")
                tg = tmp.rearrange("p (k2 d) -> p k2 d", d=D)
                s = spool.tile([128, D], bf16, tag="s")
                nc.vector.tensor_add(tg, xg[:, :, 0, :], xg[:, :, 1, :])
                nc.vector.tensor_add(s, tg[:, 0, :], tg[:, 1, :])

                # PE transposes (bf16, 1 cycle/row) + Act copies -> mts
                mts = []
                pss = [
                    ptr.tile([128, 512], bf16, tag=f"ps{h}", name=f"ps{h}")
                    for h in range(2)
                ]
                for ic in range(ICH):
                    nc.tensor.transpose(
                        pss[ic // 4][:, (ic % 4) * 128:(ic % 4 + 1) * 128],
                        s[:, ic * 128:(ic + 1) * 128],
                        ident,
                    )
                for ic in range(ICH):
                    mt = mtpool.tile([128, 128], bf16, tag=f"mt{ic}",
                                     name=f"mt{ic}")
                    nc.scalar.copy(
                        out=mt[:],
                        in_=pss[ic // 4][:, (ic % 4) * 128:(ic % 4 + 1) * 128],
                    )
                    mts.append(mt)

                # main GEMM: bars g*128..+128, all 1024 outputs in 2 halves
                osb = opool.tile([128, D], f32, tag="osb")
                for oc in range(2):
                    pm = pmm.tile([128, 512], f32, tag=f"pm{oc}",
                                  name=f"pm{oc}")
                    for ic in range(ICH):
                        nc.tensor.matmul(
                            pm[:],
                            lhsT=mts[ic][:],
                            rhs=wt_slice(ic, oc),
                            start=(ic == 0),
                            stop=False,
                        )
                    nc.tensor.matmul(
                        pm[:],
                        lhsT=fft_slice(g),
                        rhs=wtail_slice(oc),
                        start=False,
                        stop=True,
                    )
                    nc.vector.tensor_copy(osb[:, oc * 512:(oc + 1) * 512], pm[:])
                # out writes on the gpsimd SWDGE queue (third DMA path)
                nc.gpsimd.dma_start(
                    out=out[g * 128:(g + 1) * 128, :], in_=osb[:]
                )

        if reps == 1:
            body()
        else:
            # hardware loop: reps iterations without reps x instructions
            # (benchmark-only path; kernel() always uses reps=1)
            with tc.For_i(0, reps):
                body()


_NC_CACHE: bass.Bass | None = None


def _get_nc() -> bass.Bass:
    global _NC_CACHE
    if _NC_CACHE is None:
        nc = bacc.Bacc(trn_type="TRN2")
        _emit(nc)
        nc.compile()
        _NC_CACHE = nc
    return _NC_CACHE


def _host_cst(W: np.ndarray, b: np.ndarray) -> np.ndarray:
    """Pack [0.25*W1T ; W2T ; b ; fourierT ; ones ; I] into one bf16 tensor."""
    wt_aug = np.concatenate(
        [
            0.25 * np.ascontiguousarray(W[:, :D].T),
            np.ascontiguousarray(W[:, D:].T),
            np.asarray(b, np.float32)[None, :],
        ],
        axis=0,
    ).astype(np.float32)  # [1057, 1024]
    fft = _fourier_T()

    cst = np.zeros((128, CST_F), dtype=ml_dtypes.bfloat16)
    for ic in range(ICH):
        cst[:, COL_WT + ic * D:COL_WT + (ic + 1) * D] = wt_aug[
            ic * 128:(ic + 1) * 128
        ].astype(ml_dtypes.bfloat16)
    cst[0:POS + 1, COL_WTAIL:COL_WTAIL + D] = wt_aug[D:].astype(ml_dtypes.bfloat16)
    cst[0:POS + 1, COL_FFT:COL_FFT + MB] = fft.astype(ml_dtypes.bfloat16)
    cst[:, COL_ID:COL_ID + 128] = np.eye(128, dtype=np.float32).astype(
        ml_dtypes.bfloat16
    )
    return cst


def _host_inputs(beat_emb: np.ndarray, W: np.ndarray, b: np.ndarray):
    cst = _host_cst(np.asarray(W, np.float32), np.asarray(b, np.float32))
    tok = np.zeros((128, 128), np.float32)
    return [
        {
            "x": np.ascontiguousarray(beat_emb[i], dtype=np.float32),
            "cst": cst,
            "tok": tok,
        }
        for i in range(NCORES)
    ]


def _numpy_model(beat_emb: np.ndarray, W: np.ndarray, b: np.ndarray) -> np.ndarray:
    """Pure-numpy model of what the device computes (fast math check)."""
    cst = _host_cst(np.asarray(W, np.float32), np.asarray(b, np.float32))
    wt = np.concatenate(
        [
            np.concatenate(
                [
                    cst[:, COL_WT + ic * D:COL_WT + (ic + 1) * D]
                    for ic in range(ICH)
                ],
                axis=0,
            ),
            cst[0:POS + 1, COL_WTAIL:COL_WTAIL + D],
        ],
        axis=0,
    ).astype(np.float64)  # [1057, 1024]
    fft = cst[0:POS + 1, COL_FFT:COL_FFT + MB].astype(np.float64)
    outs = []
    for i in range(NCORES):
        x = np.asarray(beat_emb[i], np.float32).astype(np.float64)
        sums = x.reshape(MB, KBEATS, D).sum(axis=1)
        sums_b = sums.astype(np.float32).astype(ml_dtypes.bfloat16).astype(np.float64)
        h = np.concatenate([sums_b, fft.T], axis=1)  # [MB, 1057]
        outs.append((h @ wt).astype(np.float32))
    return np.stack(outs, 0)


def kernel(beat_emb: np.ndarray, W: np.ndarray, b: np.ndarray) -> np.ndarray:
    nc = _get_nc()
    in_maps = _host_inputs(np.asarray(beat_emb), np.asarray(W), np.asarray(b))
    res = run_bass_kernel_spmd(nc, in_maps, core_ids=list(range(NCORES)))
    return np.stack([np.asarray(res.results[i]["out"]) for i in range(NCORES)], axis=0)
